# revision 26
# baseline (speedup 1.0000x reference)
"""AttentionBlock (GroupNorm32 + 8-head global self-attention + proj + residual)
on 8 TRN2 NeuronCores, data-parallel over batch (B=8 -> 1 image per core).

v5: ACT-exp is the critical resource. The kernel keeps ScalarE busy on exp
while PE fills slack with QKV / V / PV / proj, staying HAM-warm:

  - x + pair-0 qk weights DMA'd first (split per-pair weight slices);
    GroupNorm per-tile chains as x lands; PE warmup matmuls un-throttle HAM.
  - S^T in a manual 6-bank PSUM region, 3 rotating [128,1024] slots with
    slot = (2*gmt)%3: two thirds of the m-tiles get their even/odd-head
    S tiles in adjacent slots and are exp'd in one fused [128,2048]
    ACTIVATE (amortizes the ~0.5us/instr overhead).
  - pT laid out [128, mt, hh, n] so fused exp output is contiguous.
  - PV runs one pair behind exp as dense per-(head,nhalf) groups of 8
    accumulating matmuls -> short PSUM holds; 2-bank pool shared with
    QKV/V/bc fillers.
  - Softmax denominators from the vT ones-row; normalize: denom rows
    reshaped onto 128 partitions by SBUF->SBUF DMA, [128,8] reciprocal,
    DMA back to K-rows {0,32}, K=33 broadcast matmul, one multiply.
  - proj streams per [128,512] chunk with residual into all free banks.
"""
import numpy as np

C = 512
NH = 8
D = 64
N = 1024
GROUPS = 32
GS = C // GROUPS  # 16 channels per group
EPS = 1e-5
B = 8
CT = C // 128      # 4 channel tiles (= head pairs)
MT = N // 128      # 8 m-tiles
NHF = 2            # n halves of 512

TRACE = False     # test.py flips this for profiling runs

_cache = {}


def _build():
    import concourse.bacc as bacc
    import concourse.tile as tile
    import concourse.mybir as mybir

    F32 = mybir.dt.float32
    F32R = mybir.dt.float32r
    BF16 = mybir.dt.bfloat16
    AF = mybir.ActivationFunctionType
    ALU = mybir.AluOpType
    nc = bacc.Bacc("TRN2", target_bir_lowering=False, debug=False,
                   enable_asserts=False, num_devices=1)

    x_d = nc.dram_tensor("x", [C, N], F32, kind="ExternalInput").ap()
    qkv_wT_d = nc.dram_tensor("qkv_wT", [C, 3 * C], BF16, kind="ExternalInput").ap()
    proj_wT_d = nc.dram_tensor("proj_wT", [C, C], BF16, kind="ExternalInput").ap()
    # packed per-channel params: cols 0-3 gn_w, 4-7 gn_b, 8-15 qk_bias, 16-19 proj_be
    pp_d = nc.dram_tensor("pp", [128, 20], F32, kind="ExternalInput").ap()
    sel_d = nc.dram_tensor("sel", [128, 8], F32R, kind="ExternalInput").ap()
    expd_d = nc.dram_tensor("expd", [8, 128], F32R, kind="ExternalInput").ap()
    # denominator-broadcast selector: row 0 -> cols 0-63, row 32 -> cols 64-127
    sel2_d = nc.dram_tensor("sel2", [33, 128], F32R, kind="ExternalInput").ap()
    out_d = nc.dram_tensor("out", [C, N], F32, kind="ExternalOutput").ap()

    scale = float(D) ** -0.5

    with tile.TileContext(nc) as tc:
        with tc.tile_pool(name="const", bufs=1) as const, \
             tc.tile_pool(name="big", bufs=1) as big, \
             tc.tile_pool(name="pT_pool", bufs=2) as pT_pool, \
             tc.tile_pool(name="small", bufs=2) as small, \
             tc.tile_pool(name="rsp", bufs=2) as rsp, \
             tc.tile_pool(name="outp", bufs=2) as outp, \
             tc.tile_pool(name="psb_pool", bufs=1, space="PSUM") as psb_pool, \
             tc.tile_pool(name="pm", bufs=2, space="PSUM") as pm:

            # ---- DMA in: x has priority; pair-0 qk weight slices early ----
            pp = const.tile([128, 20], F32)
            sel = const.tile([128, 8], F32R)
            expd = const.tile([8, 128], F32R)
            sel2 = const.tile([33, 128], F32R)
            rs2a = const.tile([33, 512], F32R)
            rs2b = const.tile([33, 512], F32R)
            z33 = const.tile([33, 512], F32)
            eps_t = const.tile([8, 1], F32)
            nc.vector.memset(z33, 0.0)
            nc.vector.tensor_copy(out=rs2a, in_=z33)
            nc.vector.tensor_copy(out=rs2b, in_=z33)
            nc.vector.memset(eps_t, EPS)

            # x loaded as bf16 (cast-on-DMA, gpsimd queue): halves the
            # startup-critical HBM bytes. Residual/GN in bf16 is well within
            # the error budget.
            x_sb = big.tile([128, CT, N], BF16)
            qkv_wT = const.tile([128, CT, 3 * C], BF16)
            proj_wT = const.tile([128, CT, C], BF16)

            nc.sync.dma_start(out=pp, in_=pp_d)
            nc.scalar.dma_start(out=sel, in_=sel_d)
            nc.scalar.dma_start(out=expd, in_=expd_d)
            nc.scalar.dma_start(out=sel2, in_=sel2_d)
            for ci in range(CT):
                nc.gpsimd.dma_start(out=x_sb[:, ci, :],
                                    in_=x_d[128 * ci:128 * (ci + 1), :])
            nc.sync.dma_start(
                out=qkv_wT[:, :, 0:2 * C],
                in_=qkv_wT_d[:, 0:2 * C].rearrange("(t p) o -> p t o", p=128))
            nc.sync.dma_start(
                out=qkv_wT[:, :, 2 * C:3 * C],
                in_=qkv_wT_d[:, 2 * C:3 * C].rearrange("(t p) o -> p t o", p=128))
            nc.sync.dma_start(out=proj_wT,
                              in_=proj_wT_d.rearrange("(t p) o -> p t o", p=128))

            gn_w = pp[:, 0:4]
            gn_b = pp[:, 4:8]
            qk_bias = pp[:, 8:16].rearrange("p (t o) -> p t o", o=1)
            proj_be = pp[:, 16:20].rearrange("p (t o) -> p t o", o=1)

            # ---- PSUM: 6-bank manual S region + 2-bank pm pool ----
            psb = psb_pool.tile([128, 3, N], F32)

            # ---- GroupNorm: per-tile chains, pipelined with x arrival ----
            hn = big.tile([128, CT, N], BF16)
            for ci in range(CT):
                bstats = small.tile([128, 2, 6], F32, tag="bstats")
                xv = x_sb[:, ci, :].rearrange("p (s n) -> p s n", s=2)
                for s in range(2):
                    nc.vector.bn_stats(out=bstats[:, s, :], in_=xv[:, s, :])
                mv = small.tile([128, 2], F32, tag="mv")
                nc.vector.bn_aggr(out=mv, in_=bstats)
                stat_rhs = small.tile([128, 2], F32R, tag="statr")
                nc.vector.tensor_copy(out=stat_rhs[:, 0:1], in_=mv[:, 0:1])
                nc.vector.tensor_tensor(out=stat_rhs[:, 1:2], in0=mv[:, 0:1],
                                        in1=mv[:, 0:1], op=ALU.mult)
                nc.vector.tensor_tensor(out=stat_rhs[:, 1:2], in0=stat_rhs[:, 1:2],
                                        in1=mv[:, 1:2], op=ALU.add)
                grp_ps = pm.tile([8, 2], F32, tag="pm", name=f"grp_ps{ci}")
                nc.tensor.matmul(grp_ps[:], sel, stat_rhs, start=True, stop=True)
                g2 = small.tile([8, 2], F32R, tag="g2")
                gv = small.tile([8, 1], F32, tag="gv")
                gm = small.tile([8, 1], F32, tag="gm")
                nc.vector.tensor_scalar(out=gm, in0=grp_ps[:, 0:1],
                                        scalar1=1.0 / GS, scalar2=None, op0=ALU.mult)
                nc.vector.tensor_scalar(out=gv, in0=grp_ps[:, 1:2],
                                        scalar1=1.0 / GS, scalar2=None, op0=ALU.mult)
                gm2 = small.tile([8, 1], F32, tag="gm2")
                nc.vector.tensor_tensor(out=gm2, in0=gm, in1=gm, op=ALU.mult)
                nc.vector.tensor_tensor(out=gv, in0=gv, in1=gm2, op=ALU.subtract)
                nc.scalar.activation(out=gv, in_=gv, func=AF.Sqrt,
                                     bias=eps_t, scale=1.0)
                nc.vector.reciprocal(out=gv, in_=gv)
                nc.vector.tensor_copy(out=g2[:, 0:1], in_=gm)
                nc.vector.tensor_copy(out=g2[:, 1:2], in_=gv)
                ab_ps = pm.tile([128, 2], F32, tag="pm", name=f"ab_ps{ci}")
                nc.tensor.matmul(ab_ps[:], expd, g2, start=True, stop=True)
                A = small.tile([128, 1], F32, tag="A")
                Bb = small.tile([128, 1], F32, tag="Bb")
                nc.vector.tensor_tensor(out=A, in0=ab_ps[:, 1:2],
                                        in1=gn_w[:, ci:ci + 1], op=ALU.mult)
                nc.vector.tensor_tensor(out=Bb, in0=ab_ps[:, 0:1], in1=A,
                                        op=ALU.mult)
                nc.vector.tensor_tensor(out=Bb, in0=gn_b[:, ci:ci + 1], in1=Bb,
                                        op=ALU.subtract)
                nc.vector.tensor_scalar(out=hn[:, ci, :], in0=x_sb[:, ci, :],
                                        scalar1=A, scalar2=Bb,
                                        op0=ALU.mult, op1=ALU.add)

            # ---- PE warmup: un-throttle HAM right before QKV; gated on
            # weight arrival so the 3.4us-busy window lands next to real work
            wu_ps = pm.tile([128, 512], F32, tag="pm", name="wu")
            for w in range(8):
                nc.tensor.matmul(wu_ps[:, 0:256], qkv_wT[:, 0, 0:128],
                                 qkv_wT[:, 0, 0:256], start=True, stop=True)

            # ---- SBUF state ----
            q_sb = big.tile([128, CT, N], BF16)
            k_sb = big.tile([128, CT, N], BF16)
            vT = big.tile([128, MT, NH, D + 2], BF16)
            oT = big.tile([128, CT, N], BF16)
            nc.vector.memset(vT[:, :, :, D:D + 1], 1.0)
            nc.vector.memset(vT[:, :, :, D + 1:D + 2], 0.0)

            def qkv_pair(t):
                for which in range(2):  # 0=q, 1=k
                    dest = q_sb if which == 0 else k_sb
                    bt = which * CT + t
                    for nh in range(NHF):
                        ps = pm.tile([128, 512], F32, tag="pm",
                                     name=f"qk{t}_{which}_{nh}")
                        for kt in range(CT):
                            nc.tensor.matmul(
                                ps[:],
                                qkv_wT[:, kt, 128 * bt:128 * (bt + 1)],
                                hn[:, kt, 512 * nh:512 * (nh + 1)],
                                start=(kt == 0), stop=(kt == CT - 1))
                        nc.vector.tensor_scalar(
                            out=dest[:, t, 512 * nh:512 * (nh + 1)], in0=ps[:],
                            scalar1=qk_bias[:, bt, :], scalar2=None, op0=ALU.add)

            def v_tile(mt):
                ps = pm.tile([128, 512], F32, tag="pm", name=f"v{mt}")
                for kt in range(CT):
                    nc.tensor.matmul(ps[:], hn[:, kt, 128 * mt:128 * (mt + 1)],
                                     qkv_wT[:, kt, 2 * C:3 * C],
                                     start=(kt == 0), stop=(kt == CT - 1))
                nc.vector.tensor_copy(
                    out=vT[:, mt, :, 0:D],
                    in_=ps[:].rearrange("p (h d) -> p h d", h=NH))

            pT_of = {}

            def s_exp(t):
                # S^T + exp for pair t. Global m-tile index drives the 3-slot
                # rotation of the 6-bank psb region; when the two heads' S
                # tiles land in adjacent slots, one fused [128,2048] exp.
                pT_t = pT_pool.tile([128, MT, 2, N], BF16, tag="pT",
                                    name=f"pT_{t}")
                pT_of[t] = pT_t
                for mt in range(MT):
                    gmt = MT * t + mt
                    s0 = (2 * gmt) % 3
                    s1 = (2 * gmt + 1) % 3
                    for hh, sl in ((0, s0), (1, s1)):
                        qp = hh * 64
                        for nh in range(NHF):
                            nc.tensor.matmul(
                                psb[:, sl, 512 * nh:512 * (nh + 1)],
                                k_sb[qp:qp + 64, t, 128 * mt:128 * (mt + 1)],
                                q_sb[qp:qp + 64, t, 512 * nh:512 * (nh + 1)],
                                start=True, stop=True)
                    if s1 == s0 + 1:
                        nc.scalar.activation(out=pT_t[:, mt, :, :],
                                             in_=psb[:, s0:s0 + 2, :],
                                             func=AF.Exp, scale=scale)
                    else:
                        for hh, sl in ((0, s0), (1, s1)):
                            nc.scalar.activation(out=pT_t[:, mt, hh, :],
                                                 in_=psb[:, sl, :],
                                                 func=AF.Exp, scale=scale)

            def pv_group(t, nh, hh):
                # dense 8-matmul accumulation for one (head, n-half)
                pT_t = pT_of[t]
                ps_o = pm.tile([D + 2, 512], F32, tag="pm",
                               name=f"pso_{t}_{nh}_{hh}")
                for mt in range(MT):
                    nc.tensor.matmul(ps_o[:],
                                     vT[:, mt, 2 * t + hh, :],
                                     pT_t[:, mt, hh, 512 * nh:512 * (nh + 1)],
                                     start=(mt == 0), stop=(mt == MT - 1))
                ns = slice(512 * nh, 512 * (nh + 1))
                qp = hh * 64
                rsb = rsp.tile([1, 2, 512], F32, tag="rsb", name=f"rsb_{t}_{nh}",
                               bufs=4) if hh == 0 else _rsb_of[(t, nh)]
                if hh == 0:
                    _rsb_of[(t, nh)] = rsb
                nc.vector.tensor_copy(out=oT[qp:qp + 64, t, ns],
                                      in_=ps_o[0:D, :])
                nc.vector.tensor_copy(out=rsb[0:1, hh, :],
                                      in_=ps_o[D:D + 1, :])
                if hh == 1:
                    # both denom rows present: reshape onto 128 partitions
                    rT = rsp.tile([128, 2, 4], F32, tag="rT",
                                  name=f"rT_{t}_{nh}", bufs=4)
                    for h2 in range(2):
                        nc.sync.dma_start(
                            out=rT[:, h2, :],
                            in_=rsb[0:1, h2, :].rearrange("o (p j) -> o p j",
                                                          p=128))
                    _rT_of[(t, nh)] = rT

            _rsb_of = {}
            _rT_of = {}

            def norm(t, nh):
                # deferred: recip, DMA back to K-rows, broadcast mm, multiply
                rT = _rT_of.pop((t, nh))
                _rsb_of.pop((t, nh))
                rs2 = rs2a if (2 * t + nh) % 2 == 0 else rs2b
                ns = slice(512 * nh, 512 * (nh + 1))
                nc.vector.reciprocal(out=rT, in_=rT)
                for hh in range(2):
                    nc.gpsimd.dma_start(
                        out=rs2[32 * hh:32 * hh + 1, :].rearrange(
                            "o (p j) -> o p j", p=128),
                        in_=rT[:, hh, :])
                bc_ps = pm.tile([128, 512], F32, tag="pm", name=f"bc_{t}_{nh}")
                nc.tensor.matmul(bc_ps[:], sel2, rs2, start=True, stop=True)
                nc.vector.tensor_tensor(out=oT[:, t, ns], in0=oT[:, t, ns],
                                        in1=bc_ps[:], op=ALU.mult)

            def proj_chunk(ot, nh, ps):
                ns = slice(512 * nh, 512 * (nh + 1))
                for kt in range(CT):
                    nc.tensor.matmul(ps,
                                     proj_wT[:, kt, 128 * ot:128 * (ot + 1)],
                                     oT[:, kt, ns],
                                     start=(kt == 0), stop=(kt == CT - 1))
                oc = outp.tile([128, 512], F32, tag="oc", name=f"oc{ot}_{nh}")
                nc.vector.scalar_tensor_tensor(
                    out=oc, in0=ps, scalar=proj_be[:, ot, :],
                    in1=x_sb[:, ot, ns], op0=ALU.add, op1=ALU.add)
                nc.gpsimd.dma_start(out=out_d[128 * ot:128 * (ot + 1), ns],
                                    in_=oc)

            def proj_nh(nh):
                # spread the 4 out-tiles across psb (free after last exp) + pm
                for ot in range(CT):
                    if ot < 3:
                        ps = psb[:, ot, 512 * nh:512 * (nh + 1)]
                    else:
                        ps = pm.tile([128, 512], F32, tag="pm",
                                     name=f"pr{ot}_{nh}")[:]
                    proj_chunk(ot, nh, ps)

            def pv4(t):
                for nh in range(NHF):
                    for hh in range(2):
                        pv_group(t, nh, hh)
                norm(t, 0)
                norm(t, 1)

            # ---- emission order = scheduler priority ----
            qkv_pair(0)
            s_exp(0)
            qkv_pair(1)
            for mt in range(MT):
                v_tile(mt)
            s_exp(1)
            qkv_pair(2)
            pv4(0)
            s_exp(2)
            qkv_pair(3)
            pv4(1)
            s_exp(3)
            pv4(2)
            for nh in range(NHF):
                for hh in range(2):
                    pv_group(3, nh, hh)
            norm(3, 0)
            proj_nh(0)
            norm(3, 1)
            proj_nh(1)

    nc.compile()
    return nc


def _host_prep(x, gn_w, gn_b, qkv_w, qkv_b, proj_w, proj_b):
    xf = np.ascontiguousarray(x.reshape(B, C, N), dtype=np.float32)
    import ml_dtypes
    qkv_wT = np.ascontiguousarray(qkv_w.T).astype(ml_dtypes.bfloat16)
    proj_wT = np.ascontiguousarray(proj_w.T).astype(ml_dtypes.bfloat16)
    proj_be = (proj_b + proj_w @ qkv_b[2 * C:]).astype(np.float32)
    qk_bias = np.asarray(qkv_b[:2 * C], dtype=np.float32)
    # packed per-channel params [128, 20]: col-major by channel tile
    pp = np.zeros((128, 20), np.float32)
    pp[:, 0:4] = np.asarray(gn_w, np.float32).reshape(4, 128).T
    pp[:, 4:8] = np.asarray(gn_b, np.float32).reshape(4, 128).T
    pp[:, 8:16] = qk_bias.reshape(8, 128).T
    pp[:, 16:20] = proj_be.reshape(4, 128).T
    # per-tile group selector: channel p (within tile) -> group p//16 (of 8)
    sel = (np.arange(128)[:, None] // GS == np.arange(8)[None, :]).astype(np.float32)
    expd = np.ascontiguousarray(sel.T)
    sel2 = np.zeros((33, 128), np.float32)
    sel2[0, 0:64] = 1.0
    sel2[32, 64:128] = 1.0
    shared = {
        "qkv_wT": qkv_wT, "proj_wT": proj_wT, "pp": pp,
        "sel": sel, "expd": expd, "sel2": sel2,
    }
    return [{**shared, "x": np.ascontiguousarray(xf[i])} for i in range(B)]


def kernel(x, gn_w, gn_b, qkv_w, qkv_b, proj_w, proj_b):
    from concourse import bass_utils
    in_maps = _host_prep(np.asarray(x), np.asarray(gn_w), np.asarray(gn_b),
                         np.asarray(qkv_w), np.asarray(qkv_b),
                         np.asarray(proj_w), np.asarray(proj_b))
    if "nc" not in _cache:
        _cache["nc"] = _build()
    res = bass_utils.run_bass_kernel_spmd(_cache["nc"], in_maps,
                                          core_ids=list(range(B)), trace=TRACE)
    _cache["last_result"] = res
    out = np.stack([res.results[i]["out"] for i in range(B)])
    return out.reshape(B, C, 32, 32).astype(np.float32)


# revision 27
# speedup vs baseline: 1.1589x; 1.1589x over previous
"""AttentionBlock (GroupNorm32 + 8-head global self-attention + proj + residual)
on 8 TRN2 NeuronCores, data-parallel over batch (B=8 -> 1 image per core).

v5: ACT-exp is the critical resource. The kernel keeps ScalarE busy on exp
while PE fills slack with QKV / V / PV / proj, staying HAM-warm:

  - x + pair-0 qk weights DMA'd first (split per-pair weight slices);
    GroupNorm per-tile chains as x lands; PE warmup matmuls un-throttle HAM.
  - S^T in a manual 6-bank PSUM region, 3 rotating [128,1024] slots with
    slot = (2*gmt)%3: two thirds of the m-tiles get their even/odd-head
    S tiles in adjacent slots and are exp'd in one fused [128,2048]
    ACTIVATE (amortizes the ~0.5us/instr overhead).
  - pT laid out [128, mt, hh, n] so fused exp output is contiguous.
  - PV runs one pair behind exp as dense per-(head,nhalf) groups of 8
    accumulating matmuls -> short PSUM holds; 2-bank pool shared with
    QKV/V/bc fillers.
  - Softmax denominators from the vT ones-row; normalize: denom rows
    reshaped onto 128 partitions by SBUF->SBUF DMA, [128,8] reciprocal,
    DMA back to K-rows {0,32}, K=33 broadcast matmul, one multiply.
  - proj streams per [128,512] chunk with residual into all free banks.
"""
import numpy as np

C = 512
NH = 8
D = 64
N = 1024
GROUPS = 32
GS = C // GROUPS  # 16 channels per group
EPS = 1e-5
B = 8
CT = C // 128      # 4 channel tiles (= head pairs)
MT = N // 128      # 8 m-tiles
NHF = 2            # n halves of 512

TRACE = False     # test.py flips this for profiling runs

_cache = {}


def _build():
    import concourse.bacc as bacc
    import concourse.tile as tile
    import concourse.mybir as mybir

    F32 = mybir.dt.float32
    F32R = mybir.dt.float32r
    BF16 = mybir.dt.bfloat16
    AF = mybir.ActivationFunctionType
    ALU = mybir.AluOpType
    nc = bacc.Bacc("TRN2", target_bir_lowering=False, debug=False,
                   enable_asserts=False, num_devices=1)

    x_d = nc.dram_tensor("x", [C, N], F32, kind="ExternalInput").ap()
    qkv_wT_d = nc.dram_tensor("qkv_wT", [C, 3 * C], BF16, kind="ExternalInput").ap()
    # pair-major q|k weights: col 256*t+j = q-tile-t col j (j<128) else k-tile-t
    wqkp_d = nc.dram_tensor("wqkp", [C, 2 * C], BF16, kind="ExternalInput").ap()
    proj_wT_d = nc.dram_tensor("proj_wT", [C, C], BF16, kind="ExternalInput").ap()
    # packed per-channel params: cols 0-3 gn_w, 4-7 gn_b, 8-15 qk_bias, 16-19 proj_be
    pp_d = nc.dram_tensor("pp", [128, 20], F32, kind="ExternalInput").ap()
    sel_d = nc.dram_tensor("sel", [128, 8], F32R, kind="ExternalInput").ap()
    expd_d = nc.dram_tensor("expd", [8, 128], F32R, kind="ExternalInput").ap()
    # denominator-broadcast selector: row 0 -> cols 0-63, row 32 -> cols 64-127
    sel2_d = nc.dram_tensor("sel2", [33, 128], F32R, kind="ExternalInput").ap()
    out_d = nc.dram_tensor("out", [C, N], F32, kind="ExternalOutput").ap()

    scale = float(D) ** -0.5

    with tile.TileContext(nc) as tc:
        with tc.tile_pool(name="const", bufs=1) as const, \
             tc.tile_pool(name="big", bufs=1) as big, \
             tc.tile_pool(name="pT_pool", bufs=2) as pT_pool, \
             tc.tile_pool(name="small", bufs=2) as small, \
             tc.tile_pool(name="rsp", bufs=2) as rsp, \
             tc.tile_pool(name="outp", bufs=2) as outp, \
             tc.tile_pool(name="psb_pool", bufs=1, space="PSUM") as psb_pool, \
             tc.tile_pool(name="pm", bufs=2, space="PSUM") as pm:

            # ---- DMA in: x has priority; pair-0 qk weight slices early ----
            pp = const.tile([128, 20], F32)
            sel = const.tile([128, 8], F32R)
            expd = const.tile([8, 128], F32R)
            sel2 = const.tile([33, 128], F32R)
            rs2a = const.tile([33, 512], F32R)
            rs2b = const.tile([33, 512], F32R)
            z33 = const.tile([33, 512], F32)
            eps_t = const.tile([8, 1], F32)
            nc.vector.memset(z33, 0.0)
            nc.vector.tensor_copy(out=rs2a, in_=z33)
            nc.vector.tensor_copy(out=rs2b, in_=z33)
            nc.vector.memset(eps_t, EPS)

            # qk weights pair-major ([c, pair, q|k, 128] on host) so each
            # pair's slice is one contiguous 512B/row DMA; pair 0 first.
            x_sb = big.tile([128, CT, N], F32)
            qkv_qk = const.tile([128, CT, CT, 256], BF16)
            qkv_wv = const.tile([128, CT, C], BF16)
            proj_wT = const.tile([128, CT, C], BF16)

            nc.sync.dma_start(out=pp, in_=pp_d)
            nc.scalar.dma_start(out=sel, in_=sel_d)
            nc.scalar.dma_start(out=expd, in_=expd_d)
            nc.scalar.dma_start(out=sel2, in_=sel2_d)

            def wqkp(t, eng):
                eng.dma_start(
                    out=qkv_qk[:, :, t, :],
                    in_=wqkp_d[:, 256 * t:256 * (t + 1)].rearrange(
                        "(t p) o -> p t o", p=128))

            wqkp(0, nc.sync)
            nc.sync.dma_start(out=x_sb[:, 0, :], in_=x_d[0:128, :])
            nc.scalar.dma_start(out=x_sb[:, 1, :], in_=x_d[128:256, :])
            nc.gpsimd.dma_start(out=x_sb[:, 2, :], in_=x_d[256:384, :])
            nc.gpsimd.dma_start(out=x_sb[:, 3, :], in_=x_d[384:512, :])
            wqkp(1, nc.sync)
            wqkp(2, nc.scalar)
            wqkp(3, nc.gpsimd)
            nc.sync.dma_start(
                out=qkv_wv,
                in_=qkv_wT_d[:, 2 * C:3 * C].rearrange("(t p) o -> p t o", p=128))
            nc.scalar.dma_start(out=proj_wT,
                                in_=proj_wT_d.rearrange("(t p) o -> p t o", p=128))

            gn_w = pp[:, 0:4]
            gn_b = pp[:, 4:8]
            qk_bias = pp[:, 8:16].rearrange("p (t o) -> p t o", o=1)
            proj_be = pp[:, 16:20].rearrange("p (t o) -> p t o", o=1)

            # ---- PSUM: 6-bank manual S region + 2-bank pm pool ----
            psb = psb_pool.tile([128, 3, N], F32)

            # ---- GroupNorm: per-tile stats as x lands; one batched tail ----
            hn = big.tile([128, CT, N], BF16)
            gall = small.tile([8, CT, 2], F32, tag="gall", bufs=1)
            for ci in range(CT):
                bstats = small.tile([128, 2, 6], F32, tag="bstats")
                xv = x_sb[:, ci, :].rearrange("p (s n) -> p s n", s=2)
                for s in range(2):
                    nc.vector.bn_stats(out=bstats[:, s, :], in_=xv[:, s, :])
                mv = small.tile([128, 2], F32, tag="mv")
                nc.vector.bn_aggr(out=mv, in_=bstats)
                stat_rhs = small.tile([128, 2], F32R, tag="statr")
                nc.vector.tensor_copy(out=stat_rhs[:, 0:1], in_=mv[:, 0:1])
                nc.vector.tensor_tensor(out=stat_rhs[:, 1:2], in0=mv[:, 0:1],
                                        in1=mv[:, 0:1], op=ALU.mult)
                nc.vector.tensor_tensor(out=stat_rhs[:, 1:2], in0=stat_rhs[:, 1:2],
                                        in1=mv[:, 1:2], op=ALU.add)
                grp_ps = pm.tile([8, 2], F32, tag="pm", name=f"grp_ps{ci}")
                nc.tensor.matmul(grp_ps[:], sel, stat_rhs, start=True, stop=True)
                nc.vector.tensor_scalar(out=gall[:, ci, :], in0=grp_ps[:],
                                        scalar1=1.0 / GS, scalar2=None,
                                        op0=ALU.mult)
            gmean = gall[:, :, 0:1]
            gvar = gall[:, :, 1:2]
            gm2 = small.tile([8, CT, 1], F32, tag="gm2", bufs=1)
            nc.vector.tensor_tensor(out=gm2, in0=gmean, in1=gmean, op=ALU.mult)
            nc.vector.tensor_tensor(out=gvar, in0=gvar, in1=gm2, op=ALU.subtract)
            nc.scalar.activation(out=gvar, in_=gvar, func=AF.Sqrt,
                                 bias=eps_t, scale=1.0)
            nc.vector.reciprocal(out=gvar, in_=gvar)
            g2 = small.tile([8, CT, 2], F32R, tag="g2", bufs=1)
            nc.vector.tensor_copy(out=g2, in_=gall)
            absb = small.tile([128, CT, 2], F32, tag="absb", bufs=1)
            for ci in range(CT):
                ab_ps = pm.tile([128, 2], F32, tag="pm", name=f"ab_ps{ci}")
                nc.tensor.matmul(ab_ps[:], expd, g2[:, ci, :], start=True,
                                 stop=True)
                nc.vector.tensor_copy(out=absb[:, ci, :], in_=ab_ps[:])
            A_all = small.tile([128, CT], F32, tag="A", bufs=1)
            B_all = small.tile([128, CT], F32, tag="Bb", bufs=1)
            nc.vector.tensor_tensor(out=A_all, in0=absb[:, :, 1], in1=gn_w,
                                    op=ALU.mult)
            nc.vector.tensor_tensor(out=B_all, in0=absb[:, :, 0], in1=A_all,
                                    op=ALU.mult)
            nc.vector.tensor_tensor(out=B_all, in0=gn_b, in1=B_all,
                                    op=ALU.subtract)
            for ci in range(CT):
                nc.vector.tensor_scalar(out=hn[:, ci, :], in0=x_sb[:, ci, :],
                                        scalar1=A_all[:, ci:ci + 1],
                                        scalar2=B_all[:, ci:ci + 1],
                                        op0=ALU.mult, op1=ALU.add)

            # ---- PE warmup: un-throttle HAM right before QKV; gated on
            # weight arrival so the 3.4us-busy window lands next to real work
            wu_ps = pm.tile([128, 512], F32, tag="pm", name="wu")
            for w in range(8):
                nc.tensor.matmul(wu_ps[:, 0:256], qkv_qk[:, 0, 0, 0:128],
                                 qkv_qk[:, 0, 0, 0:256], start=True, stop=True)

            # ---- SBUF state ----
            q_sb = big.tile([128, CT, N], BF16)
            k_sb = big.tile([128, CT, N], BF16)
            vT = big.tile([128, MT, NH, D + 2], BF16)
            oT = big.tile([128, CT, N], BF16)
            nc.vector.memset(vT[:, :, :, D:D + 1], 1.0)
            nc.vector.memset(vT[:, :, :, D + 1:D + 2], 0.0)

            def qkv_pair(t):
                for which in range(2):  # 0=q, 1=k
                    dest = q_sb if which == 0 else k_sb
                    bt = which * CT + t
                    for nh in range(NHF):
                        ps = pm.tile([128, 512], F32, tag="pm",
                                     name=f"qk{t}_{which}_{nh}")
                        for kt in range(CT):
                            nc.tensor.matmul(
                                ps[:],
                                qkv_qk[:, kt, t, 128 * which:128 * (which + 1)],
                                hn[:, kt, 512 * nh:512 * (nh + 1)],
                                start=(kt == 0), stop=(kt == CT - 1))
                        nc.vector.tensor_scalar(
                            out=dest[:, t, 512 * nh:512 * (nh + 1)], in0=ps[:],
                            scalar1=qk_bias[:, bt, :], scalar2=None, op0=ALU.add)

            def v_tile(mt):
                ps = pm.tile([128, 512], F32, tag="pm", name=f"v{mt}")
                for kt in range(CT):
                    nc.tensor.matmul(ps[:], hn[:, kt, 128 * mt:128 * (mt + 1)],
                                     qkv_wv[:, kt, :],
                                     start=(kt == 0), stop=(kt == CT - 1))
                nc.vector.tensor_copy(
                    out=vT[:, mt, :, 0:D],
                    in_=ps[:].rearrange("p (h d) -> p h d", h=NH))

            pT_of = {}

            def s_exp(t):
                # S^T + exp for pair t. Global m-tile index drives the 3-slot
                # rotation of the 6-bank psb region; when the two heads' S
                # tiles land in adjacent slots, one fused [128,2048] exp.
                pT_t = pT_pool.tile([128, MT, 2, N], BF16, tag="pT",
                                    name=f"pT_{t}")
                pT_of[t] = pT_t
                for mt in range(MT):
                    gmt = MT * t + mt
                    s0 = (2 * gmt) % 3
                    s1 = (2 * gmt + 1) % 3
                    for hh, sl in ((0, s0), (1, s1)):
                        qp = hh * 64
                        for nh in range(NHF):
                            nc.tensor.matmul(
                                psb[:, sl, 512 * nh:512 * (nh + 1)],
                                k_sb[qp:qp + 64, t, 128 * mt:128 * (mt + 1)],
                                q_sb[qp:qp + 64, t, 512 * nh:512 * (nh + 1)],
                                start=True, stop=True)
                    if s1 == s0 + 1:
                        nc.scalar.activation(out=pT_t[:, mt, :, :],
                                             in_=psb[:, s0:s0 + 2, :],
                                             func=AF.Exp, scale=scale)
                    else:
                        for hh, sl in ((0, s0), (1, s1)):
                            nc.scalar.activation(out=pT_t[:, mt, hh, :],
                                                 in_=psb[:, sl, :],
                                                 func=AF.Exp, scale=scale)

            def pv_group(t, nh, hh):
                # dense 8-matmul accumulation for one (head, n-half)
                pT_t = pT_of[t]
                ps_o = pm.tile([D + 2, 512], F32, tag="pm",
                               name=f"pso_{t}_{nh}_{hh}")
                for mt in range(MT):
                    nc.tensor.matmul(ps_o[:],
                                     vT[:, mt, 2 * t + hh, :],
                                     pT_t[:, mt, hh, 512 * nh:512 * (nh + 1)],
                                     start=(mt == 0), stop=(mt == MT - 1))
                ns = slice(512 * nh, 512 * (nh + 1))
                qp = hh * 64
                rsb = rsp.tile([1, 2, 512], F32, tag="rsb", name=f"rsb_{t}_{nh}",
                               bufs=4) if hh == 0 else _rsb_of[(t, nh)]
                if hh == 0:
                    _rsb_of[(t, nh)] = rsb
                nc.vector.tensor_copy(out=oT[qp:qp + 64, t, ns],
                                      in_=ps_o[0:D, :])
                nc.vector.tensor_copy(out=rsb[0:1, hh, :],
                                      in_=ps_o[D:D + 1, :])
                if hh == 1:
                    # both denom rows present: reshape onto 128 partitions
                    rT = rsp.tile([128, 2, 4], F32, tag="rT",
                                  name=f"rT_{t}_{nh}", bufs=4)
                    for h2 in range(2):
                        nc.sync.dma_start(
                            out=rT[:, h2, :],
                            in_=rsb[0:1, h2, :].rearrange("o (p j) -> o p j",
                                                          p=128))
                    _rT_of[(t, nh)] = rT

            _rsb_of = {}
            _rT_of = {}

            def norm(t, nh):
                # deferred: recip, DMA back to K-rows, broadcast mm, multiply
                rT = _rT_of.pop((t, nh))
                _rsb_of.pop((t, nh))
                rs2 = rs2a if (2 * t + nh) % 2 == 0 else rs2b
                ns = slice(512 * nh, 512 * (nh + 1))
                nc.vector.reciprocal(out=rT, in_=rT)
                for hh in range(2):
                    nc.gpsimd.dma_start(
                        out=rs2[32 * hh:32 * hh + 1, :].rearrange(
                            "o (p j) -> o p j", p=128),
                        in_=rT[:, hh, :])
                bc_ps = pm.tile([128, 512], F32, tag="pm", name=f"bc_{t}_{nh}")
                nc.tensor.matmul(bc_ps[:], sel2, rs2, start=True, stop=True)
                nc.vector.tensor_tensor(out=oT[:, t, ns], in0=oT[:, t, ns],
                                        in1=bc_ps[:], op=ALU.mult)

            def proj_chunk(ot, nh, ps):
                ns = slice(512 * nh, 512 * (nh + 1))
                for kt in range(CT):
                    nc.tensor.matmul(ps,
                                     proj_wT[:, kt, 128 * ot:128 * (ot + 1)],
                                     oT[:, kt, ns],
                                     start=(kt == 0), stop=(kt == CT - 1))
                oc = outp.tile([128, 512], F32, tag="oc", name=f"oc{ot}_{nh}")
                nc.vector.scalar_tensor_tensor(
                    out=oc, in0=ps, scalar=proj_be[:, ot, :],
                    in1=x_sb[:, ot, ns], op0=ALU.add, op1=ALU.add)
                nc.gpsimd.dma_start(out=out_d[128 * ot:128 * (ot + 1), ns],
                                    in_=oc)

            def proj_nh(nh):
                # spread the 4 out-tiles across psb (free after last exp) + pm
                for ot in range(CT):
                    if ot < 3:
                        ps = psb[:, ot, 512 * nh:512 * (nh + 1)]
                    else:
                        ps = pm.tile([128, 512], F32, tag="pm",
                                     name=f"pr{ot}_{nh}")[:]
                    proj_chunk(ot, nh, ps)

            def pv4(t):
                for nh in range(NHF):
                    for hh in range(2):
                        pv_group(t, nh, hh)
                norm(t, 0)
                norm(t, 1)

            # ---- emission order = scheduler priority ----
            qkv_pair(0)
            s_exp(0)
            qkv_pair(1)
            for mt in range(MT):
                v_tile(mt)
            s_exp(1)
            qkv_pair(2)
            pv4(0)
            s_exp(2)
            qkv_pair(3)
            pv4(1)
            s_exp(3)
            pv4(2)
            for nh in range(NHF):
                for hh in range(2):
                    pv_group(3, nh, hh)
            norm(3, 0)
            proj_nh(0)
            norm(3, 1)
            proj_nh(1)

    nc.compile()
    return nc


def _host_prep(x, gn_w, gn_b, qkv_w, qkv_b, proj_w, proj_b):
    xf = np.ascontiguousarray(x.reshape(B, C, N), dtype=np.float32)
    import ml_dtypes
    qkv_wT = np.ascontiguousarray(qkv_w.T).astype(ml_dtypes.bfloat16)
    proj_wT = np.ascontiguousarray(proj_w.T).astype(ml_dtypes.bfloat16)
    proj_be = (proj_b + proj_w @ qkv_b[2 * C:]).astype(np.float32)
    qk_bias = np.asarray(qkv_b[:2 * C], dtype=np.float32)
    # packed per-channel params [128, 20]: col-major by channel tile
    pp = np.zeros((128, 20), np.float32)
    pp[:, 0:4] = np.asarray(gn_w, np.float32).reshape(4, 128).T
    pp[:, 4:8] = np.asarray(gn_b, np.float32).reshape(4, 128).T
    pp[:, 8:16] = qk_bias.reshape(8, 128).T
    pp[:, 16:20] = proj_be.reshape(4, 128).T
    # per-tile group selector: channel p (within tile) -> group p//16 (of 8)
    sel = (np.arange(128)[:, None] // GS == np.arange(8)[None, :]).astype(np.float32)
    expd = np.ascontiguousarray(sel.T)
    sel2 = np.zeros((33, 128), np.float32)
    sel2[0, 0:64] = 1.0
    sel2[32, 64:128] = 1.0
    wqkp = np.zeros((C, 2 * C), qkv_wT.dtype)
    for t in range(4):
        wqkp[:, 256 * t:256 * t + 128] = qkv_wT[:, 128 * t:128 * (t + 1)]
        wqkp[:, 256 * t + 128:256 * (t + 1)] = qkv_wT[:, C + 128 * t:C + 128 * (t + 1)]
    shared = {
        "qkv_wT": qkv_wT, "proj_wT": proj_wT, "pp": pp, "wqkp": wqkp,
        "sel": sel, "expd": expd, "sel2": sel2,
    }
    return [{**shared, "x": np.ascontiguousarray(xf[i])} for i in range(B)]


def kernel(x, gn_w, gn_b, qkv_w, qkv_b, proj_w, proj_b):
    from concourse import bass_utils
    in_maps = _host_prep(np.asarray(x), np.asarray(gn_w), np.asarray(gn_b),
                         np.asarray(qkv_w), np.asarray(qkv_b),
                         np.asarray(proj_w), np.asarray(proj_b))
    if "nc" not in _cache:
        _cache["nc"] = _build()
    res = bass_utils.run_bass_kernel_spmd(_cache["nc"], in_maps,
                                          core_ids=list(range(B)), trace=TRACE)
    _cache["last_result"] = res
    out = np.stack([res.results[i]["out"] for i in range(B)])
    return out.reshape(B, C, 32, 32).astype(np.float32)
